# revision 3
# baseline (speedup 1.0000x reference)
"""Trainium2 Bass kernel for nn_D1Layer_32246614458525 (vq_codebook).

Algorithm notes
---------------
The reference quantizes every scalar t of x against a 256-entry codebook of
8-dim embeddings, where the query features are [t, t^2, ..., t^8] (a 1-D
moment curve).  The nearest-codeword index as a function of t is therefore
piecewise-constant in t with very few pieces.  kernel() derives the exact
fp32 decision thresholds from emb_W alone (host-side weight preprocessing),
and the device computes the VQ as a branch-free sum of threshold
comparisons, followed by the 6-layer relu MLP.

Sharding: data-parallel over the batch.  q[b, d] = ind(x[(d*B + b) // D,
(d*B + b) % D]) with B = 2*D, so q[512u+v, d] = ind(x[2d+u, v]).  Core
c = 4u + w receives x[u::2, 128w:128w+128] ([512, 128]) and computes output
rows 512u + [128w, 128w+128) directly — the reference's reshape/transpose
permutation is absorbed into the sharding, and the elementwise VQ of that
slice *is* the transposed L1 activation (d on partitions), so no on-device
transpose is needed before layer 1.

MLP: weights are shipped/computed in fp16 (fp32 PSUM accumulation, measured
rel absmax error ~1e-3 end to end); activations flow as fp16.  Matmuls are
"form 1": lhsT = transposed activations (stationary), rhs = pre-transposed
weights (moving, N=512).  Between layers the [128b, 2048f] activation is
re-transposed to [f, b] with 16 PE transposes.  Biases are added via K=1
matmuls into the same PSUM accumulation group.
"""

import numpy as np

from concourse import bacc, bass, mybir, tile
from concourse.bass_utils import run_bass_kernel_spmd

F32 = mybir.dt.float32
F16 = mybir.dt.float16

E = 8            # embedding dim / polynomial degree
B, D_IN, H, D_OUT = 1024, 512, 2048, 512
N_CORES = 8
BPC = B // N_CORES           # batch rows per core (128)
KC_H = H // 128              # contract chunks for H (16)
FO_H = H // 512              # 512-wide output blocks for H (4)


# --------------------------------------------------------------------------
# Host-side VQ threshold derivation (uses emb_W only)
# --------------------------------------------------------------------------

def _np_argmin_fp32(ts_f32: np.ndarray, emb_W: np.ndarray) -> np.ndarray:
    """fp32 emulation of the reference's nearest-codeword argmin."""
    e = np.arange(1, E + 1, dtype=np.float32)
    xr = ts_f32[:, None] ** e[None, :]
    sm = xr.sum(1, keepdims=True, dtype=np.float32)
    emb = (emb_W.astype(np.float32) ** 2).sum(1)[None, :]
    d = sm + emb - np.float32(2.0) * (xr @ emb_W.T.astype(np.float32))
    return np.argmin(d, axis=1)


def build_vq_thresholds(emb_W: np.ndarray, lo=-8.0, hi=8.0, coarse=200_001):
    """Find the exact fp32 decision thresholds of the codebook over t.

    Returns (thetas fp32 [J], region_ids [J+1]): ind(t) = region_ids[#(t >=
    theta_j)].  Each theta is an exact fp32 boundary of the reference's fp32
    argmin (t < theta -> left id, t >= theta -> right id).
    """
    W = emb_W.astype(np.float64)
    c = (W * W).sum(1)

    grid = np.linspace(lo, hi, coarse)
    winners = np.empty(coarse, dtype=np.int64)
    CH = 100_000
    for i in range(0, coarse, CH):
        g = grid[i:i + CH]
        P = np.stack([g ** e for e in range(1, E + 1)], axis=1)
        G = c[None, :] - 2.0 * (P @ W.T)
        winners[i:i + CH] = np.argmin(G, axis=1)

    chg = np.nonzero(np.diff(winners))[0]
    region_ids = np.concatenate([[winners[0]], winners[chg + 1]]).astype(np.int64)

    thetas = []
    for j, ci in enumerate(chg):
        a, b = region_ids[j], region_ids[j + 1]
        lo_t = np.float32(grid[ci])
        hi_t = np.float32(grid[ci + 1])
        assert _np_argmin_fp32(np.array([lo_t]), emb_W)[0] == a
        assert _np_argmin_fp32(np.array([hi_t]), emb_W)[0] == b
        while True:
            mid = np.float32((lo_t.astype(np.float64) + hi_t.astype(np.float64)) / 2)
            if mid == lo_t or mid == hi_t:
                break
            if _np_argmin_fp32(np.array([mid]), emb_W)[0] == a:
                lo_t = mid
            else:
                hi_t = mid
        thetas.append(hi_t)
    return np.array(thetas, dtype=np.float32), region_ids


# --------------------------------------------------------------------------
# Bass program
# --------------------------------------------------------------------------

def build_program(thetas: np.ndarray, region_ids: np.ndarray):
    nc = bacc.Bacc(
        "TRN2", target_bir_lowering=False, debug=False, enable_asserts=False
    )

    x_d = nc.dram_tensor("x", [D_IN, BPC], F32, kind="ExternalInput")
    w1_d = nc.dram_tensor("w1t", [D_IN, H], F16, kind="ExternalInput")
    wh_d = nc.dram_tensor("wht", [H, H], F16, kind="ExternalInput")
    wo_d = nc.dram_tensor("wot", [H, D_OUT], F16, kind="ExternalInput")
    b1_d = nc.dram_tensor("b1", [1, H], F16, kind="ExternalInput")
    bh_d = nc.dram_tensor("bh", [1, H], F16, kind="ExternalInput")
    bo_d = nc.dram_tensor("bo", [1, D_OUT], F16, kind="ExternalInput")
    out_d = nc.dram_tensor("out", [BPC, D_OUT], F32, kind="ExternalOutput")

    ident_d = nc.inline_tensor(np.eye(128, dtype=np.float16), name="ident")
    ones_d = nc.inline_tensor(np.ones((1, 128), dtype=np.float16), name="ones")

    base = float(region_ids[0])
    deltas = np.diff(region_ids).astype(np.float64)
    J = len(thetas)

    with tile.TileContext(nc) as tc:
        with (
            tc.tile_pool(name="wts", bufs=1) as wts,
            tc.tile_pool(name="act", bufs=2) as act,
            tc.tile_pool(name="ph", bufs=1, space="PSUM") as php,
            tc.tile_pool(name="pt", bufs=4, space="PSUM") as ptp,
        ):
            # ---- DMA inputs (program order ~ arrival priority) ----
            xs = act.tile([128, D_IN], F32, tag="xs")
            for k in range(4):
                nc.sync.dma_start(
                    out=xs[:, 128 * k:128 * (k + 1)],
                    in_=x_d[128 * k:128 * (k + 1), :],
                )
            w1 = []
            for k in range(4):
                t_ = wts.tile([128, H], F16, tag=f"w1_{k}")
                nc.sync.dma_start(out=t_[:], in_=w1_d[128 * k:128 * (k + 1), :])
                w1.append(t_)
            b1s = wts.tile([1, H], F16, tag="b1s")
            nc.sync.dma_start(out=b1s[:], in_=b1_d[:])
            bhs = wts.tile([1, H], F16, tag="bhs")
            nc.sync.dma_start(out=bhs[:], in_=bh_d[:])
            ones_s = wts.tile([1, 128], F16, tag="ones")
            nc.sync.dma_start(out=ones_s[:], in_=ones_d.ap())
            ident_s = wts.tile([128, 128], F16, tag="ident")
            nc.sync.dma_start(out=ident_s[:], in_=ident_d.ap())

            wh = []
            for k in range(KC_H):
                t_ = wts.tile([128, H], F16, tag=f"wh_{k}")
                nc.sync.dma_start(out=t_[:], in_=wh_d[128 * k:128 * (k + 1), :])
                wh.append(t_)
            wo = []
            for k in range(KC_H):
                t_ = wts.tile([128, D_OUT], F16, tag=f"wo_{k}")
                nc.sync.dma_start(out=t_[:], in_=wo_d[128 * k:128 * (k + 1), :])
                wo.append(t_)
            bos = wts.tile([1, D_OUT], F16, tag="bos")
            nc.sync.dma_start(out=bos[:], in_=bo_d[:])

            # ---- VQ: qT[d, v] = ind(x[d, v]) via threshold sums ----
            acc_a = act.tile([128, D_IN], F32, tag="vq_a")
            acc_b = act.tile([128, D_IN], F32, tag="vq_b")
            tmp = act.tile([128, D_IN], F32, tag="vq_t")
            nc.vector.tensor_scalar(
                acc_a[:], xs[:], float(thetas[0]), float(deltas[0]),
                mybir.AluOpType.is_ge, mybir.AluOpType.mult,
            )
            cur, nxt = acc_a, acc_b
            for j in range(1, J):
                nc.vector.tensor_scalar(
                    tmp[:], xs[:], float(thetas[j]), float(deltas[j]),
                    mybir.AluOpType.is_ge, mybir.AluOpType.mult,
                )
                nc.vector.tensor_tensor(
                    nxt[:], cur[:], tmp[:], mybir.AluOpType.add
                )
                cur, nxt = nxt, cur
            qT = act.tile([128, D_IN], F16, tag="qT")
            nc.vector.tensor_scalar_add(qT[:], cur[:], base)

            # ---- L1: h = relu(q @ W_in.T + b_in) ----
            # k-major accumulation so compute starts as soon as chunks land
            phs = [
                php.tile([128, 512], F32, tag=f"ph{fo}", name=f"ph_l1_{fo}")
                for fo in range(FO_H)
            ]
            for k in range(4):
                for fo in range(FO_H):
                    nc.tensor.matmul(
                        phs[fo][:],
                        lhsT=qT[:, 128 * k:128 * (k + 1)],
                        rhs=w1[k][:, 512 * fo:512 * (fo + 1)],
                        start=(k == 0), stop=False,
                    )
            h = act.tile([128, H], F16, tag="h")
            for fo in range(FO_H):
                nc.tensor.matmul(
                    phs[fo][:], lhsT=ones_s[:],
                    rhs=b1s[:, 512 * fo:512 * (fo + 1)],
                    start=False, stop=True,
                )
                nc.scalar.activation(
                    h[:, 512 * fo:512 * (fo + 1)], phs[fo][:],
                    mybir.ActivationFunctionType.Relu,
                )

            # ---- L2..L5: h = relu(h @ W_h.T + b_h), shared weights ----
            for _layer in range(4):
                hT = act.tile([128, H], F16, tag="hT")
                for p in range(KC_H):
                    pt = ptp.tile([128, 128], F16, tag="pt")
                    nc.tensor.transpose(
                        pt[:], h[:, 128 * p:128 * (p + 1)], ident_s[:]
                    )
                    nc.vector.tensor_copy(hT[:, 128 * p:128 * (p + 1)], pt[:])
                phs = [
                    php.tile([128, 512], F32, tag=f"ph{fo}",
                             name=f"ph_l{_layer}_{fo}")
                    for fo in range(FO_H)
                ]
                for k in range(KC_H):
                    for fo in range(FO_H):
                        nc.tensor.matmul(
                            phs[fo][:],
                            lhsT=hT[:, 128 * k:128 * (k + 1)],
                            rhs=wh[k][:, 512 * fo:512 * (fo + 1)],
                            start=(k == 0), stop=False,
                        )
                h = act.tile([128, H], F16, tag="h")
                for fo in range(FO_H):
                    nc.tensor.matmul(
                        phs[fo][:], lhsT=ones_s[:],
                        rhs=bhs[:, 512 * fo:512 * (fo + 1)],
                        start=False, stop=True,
                    )
                    nc.scalar.activation(
                        h[:, 512 * fo:512 * (fo + 1)], phs[fo][:],
                        mybir.ActivationFunctionType.Relu,
                    )

            # ---- L6: f = relu(h @ W_out.T + b_out) ----
            hT = act.tile([128, H], F16, tag="hT")
            for p in range(KC_H):
                pt = ptp.tile([128, 128], F16, tag="pt")
                nc.tensor.transpose(
                    pt[:], h[:, 128 * p:128 * (p + 1)], ident_s[:]
                )
                nc.vector.tensor_copy(hT[:, 128 * p:128 * (p + 1)], pt[:])
            pho = php.tile([128, D_OUT], F32, tag="ph0")
            for k in range(KC_H):
                nc.tensor.matmul(
                    pho[:], lhsT=hT[:, 128 * k:128 * (k + 1)], rhs=wo[k][:],
                    start=(k == 0), stop=False,
                )
            nc.tensor.matmul(
                pho[:], lhsT=ones_s[:], rhs=bos[:], start=False, stop=True
            )
            fo_s = act.tile([128, D_OUT], F32, tag="fout")
            nc.scalar.activation(
                fo_s[:], pho[:], mybir.ActivationFunctionType.Relu
            )
            nc.sync.dma_start(out=out_d[:], in_=fo_s[:])

    nc.compile()
    return nc


_PROGRAM_CACHE: dict[bytes, object] = {}


def _get_program(thetas: np.ndarray, region_ids: np.ndarray):
    key = thetas.tobytes() + region_ids.tobytes()
    if key not in _PROGRAM_CACHE:
        _PROGRAM_CACHE[key] = build_program(thetas, region_ids)
    return _PROGRAM_CACHE[key]


# --------------------------------------------------------------------------
# Entry point
# --------------------------------------------------------------------------

def kernel(x, emb_W, W_in, b_in, W_h, b_h, W_out, b_out):
    x = np.asarray(x, dtype=np.float32)
    emb_W = np.asarray(emb_W, dtype=np.float32)

    thetas, region_ids = build_vq_thresholds(emb_W)
    nc = _get_program(thetas, region_ids)

    w1t = np.ascontiguousarray(np.asarray(W_in).T).astype(np.float16)
    wht = np.ascontiguousarray(np.asarray(W_h).T).astype(np.float16)
    wot = np.ascontiguousarray(np.asarray(W_out).T).astype(np.float16)
    b1 = np.asarray(b_in).reshape(1, H).astype(np.float16)
    bh = np.asarray(b_h).reshape(1, H).astype(np.float16)
    bo = np.asarray(b_out).reshape(1, D_OUT).astype(np.float16)

    in_maps = []
    for c in range(N_CORES):
        u, w = c // 4, c % 4
        x_core = np.ascontiguousarray(x[u::2, 128 * w:128 * (w + 1)])
        in_maps.append({
            "x": x_core, "w1t": w1t, "wht": wht, "wot": wot,
            "b1": b1, "bh": bh, "bo": bo,
        })

    res = run_bass_kernel_spmd(nc, in_maps, core_ids=list(range(N_CORES)))
    f = np.concatenate(
        [np.asarray(res.results[c]["out"]) for c in range(N_CORES)], axis=0
    ).astype(np.float32)
    return (f, np.float32(0.0))


# revision 8
# speedup vs baseline: 1.0208x; 1.0208x over previous
"""Trainium2 Bass kernel for nn_D1Layer_32246614458525 (vq_codebook).

Algorithm notes
---------------
The reference quantizes every scalar t of x against a 256-entry codebook of
8-dim embeddings, where the query features are [t, t^2, ..., t^8] (a 1-D
moment curve).  The nearest-codeword index as a function of t is therefore
piecewise-constant in t with very few pieces (8 thresholds for this
codebook).  kernel() derives the exact fp32 decision thresholds from emb_W
alone (host-side weight preprocessing), and the device computes the VQ as a
branch-free sum of threshold sign() comparisons, followed by the 6-layer
relu MLP.

Sharding: data-parallel over the batch.  q[b, d] = ind(x[(d*B + b) // D,
(d*B + b) % D]) with B = 2*D, so q[512u+v, d] = ind(x[2d+u, v]).  Core
c = 4u + w receives x[u::2, 128w:128w+128] ([512, 128]) and computes output
rows 512u + [128w, 128w+128) directly — the reference's reshape/transpose
permutation is absorbed into the sharding, and the elementwise VQ of that
slice *is* the transposed L1 activation (d on partitions).

MLP: fp16 weights/activations, fp32 PSUM accumulation (measured rel absmax
error ~1e-3 end to end).  Matmuls: lhsT = transposed activations
(stationary), rhs = pre-transposed weights (moving, N=512).  Between layers
the [128b, 2048f] pre-activation is copied to SBUF raw, PE-transposed, and
the bias-add + relu are fused into the per-tile PSUM->SBUF ACT op after the
transpose (bias is per-partition there).  The VQ base index constant is
folded into the layer-1 bias host-side.
"""

import numpy as np

from concourse import bacc, bass, mybir, tile
from concourse.bass_utils import run_bass_kernel_spmd

F32 = mybir.dt.float32
F16 = mybir.dt.float16

E = 8            # embedding dim / polynomial degree
B, D_IN, H, D_OUT = 1024, 512, 2048, 512
N_CORES = 8
BPC = B // N_CORES           # batch rows per core (128)
KC_H = H // 128              # contract chunks for H (16)
FO_H = H // 512              # 512-wide output blocks for H (4)
N_WARMUP_MM = 48             # PE warmup matmuls (HAM un-throttle)


# --------------------------------------------------------------------------
# Host-side VQ threshold derivation (uses emb_W only)
# --------------------------------------------------------------------------

def _np_argmin_fp32(ts_f32: np.ndarray, emb_W: np.ndarray) -> np.ndarray:
    """fp32 emulation of the reference's nearest-codeword argmin."""
    e = np.arange(1, E + 1, dtype=np.float32)
    xr = ts_f32[:, None] ** e[None, :]
    sm = xr.sum(1, keepdims=True, dtype=np.float32)
    emb = (emb_W.astype(np.float32) ** 2).sum(1)[None, :]
    d = sm + emb - np.float32(2.0) * (xr @ emb_W.T.astype(np.float32))
    return np.argmin(d, axis=1)


def build_vq_thresholds(emb_W: np.ndarray, lo=-8.0, hi=8.0, coarse=200_001):
    """Find the exact fp32 decision thresholds of the codebook over t.

    Returns (thetas fp32 [J], region_ids [J+1]): ind(t) = region_ids[#(t >=
    theta_j)].  Each theta is an exact fp32 boundary of the reference's fp32
    argmin (t < theta -> left id, t >= theta -> right id).
    """
    W = emb_W.astype(np.float64)
    c = (W * W).sum(1)

    grid = np.linspace(lo, hi, coarse)
    winners = np.empty(coarse, dtype=np.int64)
    CH = 100_000
    for i in range(0, coarse, CH):
        g = grid[i:i + CH]
        P = np.stack([g ** e for e in range(1, E + 1)], axis=1)
        G = c[None, :] - 2.0 * (P @ W.T)
        winners[i:i + CH] = np.argmin(G, axis=1)

    chg = np.nonzero(np.diff(winners))[0]
    region_ids = np.concatenate([[winners[0]], winners[chg + 1]]).astype(np.int64)

    thetas = []
    for j, ci in enumerate(chg):
        a, b = region_ids[j], region_ids[j + 1]
        lo_t = np.float32(grid[ci])
        hi_t = np.float32(grid[ci + 1])
        assert _np_argmin_fp32(np.array([lo_t]), emb_W)[0] == a
        assert _np_argmin_fp32(np.array([hi_t]), emb_W)[0] == b
        while True:
            mid = np.float32((lo_t.astype(np.float64) + hi_t.astype(np.float64)) / 2)
            if mid == lo_t or mid == hi_t:
                break
            if _np_argmin_fp32(np.array([mid]), emb_W)[0] == a:
                lo_t = mid
            else:
                hi_t = mid
        thetas.append(hi_t)
    return np.array(thetas, dtype=np.float32), region_ids


# --------------------------------------------------------------------------
# Bass program
# --------------------------------------------------------------------------

def build_program(thetas: np.ndarray, region_ids: np.ndarray):
    """VQ via sign(): q0 = sum_j (delta_j/2) * sign(t - theta_j); the
    constant base' = ids[0] + sum_j delta_j/2 is folded into the layer-1
    bias host-side (kernel() computes b1_eff)."""
    nc = bacc.Bacc(
        "TRN2", target_bir_lowering=False, debug=False, enable_asserts=False
    )

    x_d = nc.dram_tensor("x", [D_IN, BPC], F32, kind="ExternalInput")
    w1_d = nc.dram_tensor("w1t", [D_IN, H], F16, kind="ExternalInput")
    wh_d = nc.dram_tensor("wht", [H, H], F16, kind="ExternalInput")
    wo_d = nc.dram_tensor("wot", [H, D_OUT], F16, kind="ExternalInput")
    # biases as per-partition column tiles [128, n_chunks] fp32
    b1_d = nc.dram_tensor("b1c", [128, KC_H], F32, kind="ExternalInput")
    bh_d = nc.dram_tensor("bhc", [128, KC_H], F32, kind="ExternalInput")
    bo_d = nc.dram_tensor("bo", [1, D_OUT], F16, kind="ExternalInput")
    out_d = nc.dram_tensor("out", [BPC, D_OUT], F32, kind="ExternalOutput")

    ident_d = nc.inline_tensor(np.eye(128, dtype=np.float16), name="ident")
    ones_d = nc.inline_tensor(np.ones((1, 128), dtype=np.float16), name="ones")
    negth_d = nc.inline_tensor(
        np.tile(-thetas.astype(np.float32)[None, :], (128, 1)), name="negth"
    )

    deltas = np.diff(region_ids).astype(np.float64)
    J = len(thetas)

    with tile.TileContext(nc) as tc:
        with (
            tc.tile_pool(name="wts", bufs=1) as wts,
            tc.tile_pool(name="act", bufs=2) as act,
            tc.tile_pool(name="ph", bufs=1, space="PSUM") as php,
            tc.tile_pool(name="pt", bufs=3, space="PSUM") as ptp,
        ):
            # ---- DMA inputs (program order ~ arrival priority) ----
            xs = act.tile([128, D_IN], F32, tag="xs")
            nc.sync.dma_start(
                out=xs[:].rearrange("p (k v) -> p k v", k=4),
                in_=x_d.ap().rearrange("(k p) v -> p k v", p=128),
            )
            ident_s = wts.tile([128, 128], F16, tag="ident")
            nc.sync.dma_start(out=ident_s[:], in_=ident_d.ap())
            ones_s = wts.tile([1, 128], F16, tag="ones")
            nc.sync.dma_start(out=ones_s[:], in_=ones_d.ap())
            negth = wts.tile([128, len(thetas)], F32, tag="negth")
            nc.sync.dma_start(out=negth[:], in_=negth_d.ap())
            b1c = wts.tile([128, KC_H], F32, tag="b1c")
            nc.sync.dma_start(out=b1c[:], in_=b1_d.ap())
            bhc = wts.tile([128, KC_H], F32, tag="bhc")
            nc.sync.dma_start(out=bhc[:], in_=bh_d.ap())

            # W_in: 2 x 1MB transfers, each [256 rows, 2048] -> [128, 2, 2048]
            w1 = []
            for g in range(2):
                t_ = wts.tile([128, 2 * H], F16, tag=f"w1_{g}")
                nc.sync.dma_start(
                    out=t_[:].rearrange("p (j f) -> p j f", j=2),
                    in_=w1_d[256 * g:256 * (g + 1), :].rearrange(
                        "(j p) f -> p j f", p=128
                    ),
                )
                w1.append(t_)

            # W_h: 8 x 1MB transfers
            wh = []
            for g in range(8):
                t_ = wts.tile([128, 2 * H], F16, tag=f"wh_{g}")
                nc.sync.dma_start(
                    out=t_[:].rearrange("p (j f) -> p j f", j=2),
                    in_=wh_d[256 * g:256 * (g + 1), :].rearrange(
                        "(j p) f -> p j f", p=128
                    ),
                )
                wh.append(t_)

            # W_out: 2 x 1MB transfers ([128, 8, 512]); b_out last
            wo = []
            for g in range(2):
                t_ = wts.tile([128, 8 * D_OUT], F16, tag=f"wo_{g}")
                nc.sync.dma_start(
                    out=t_[:].rearrange("p (j f) -> p j f", j=8),
                    in_=wo_d[1024 * g:1024 * (g + 1), :].rearrange(
                        "(j p) f -> p j f", p=128
                    ),
                )
                wo.append(t_)
            bos = wts.tile([1, D_OUT], F16, tag="bos")
            nc.sync.dma_start(out=bos[:], in_=bo_d.ap())

            def w1_chunk(k):     # lhs-contract chunk k of W_in.T, [128, H]
                return w1[k // 2][:, (k % 2) * H:(k % 2 + 1) * H]

            def wh_chunk(k):
                return wh[k // 2][:, (k % 2) * H:(k % 2 + 1) * H]

            def wo_chunk(k):     # [128, 512]
                return wo[k // 8][:, (k % 8) * D_OUT:(k % 8 + 1) * D_OUT]

            # ---- PE warmup: keep HAM un-throttled before L1 arrives ----
            pw = ptp.tile([128, 128], F16, tag="pt", name="pwarm")
            for i in range(N_WARMUP_MM):
                nc.tensor.transpose(pw[:], ident_s[:], ident_s[:])

            # ---- VQ: q0 = sum_j (delta_j/2)*sign(t - theta_j) ----
            sgn = [
                act.tile([128, D_IN], F16, tag=f"sg{j % 2}", name=f"sgn{j}")
                for j in range(J)
            ]
            for j in range(J):
                nc.scalar.activation(
                    sgn[j][:], xs[:], mybir.ActivationFunctionType.Sign,
                    bias=negth[:, j:j + 1],
                )
            accs = [
                act.tile([128, D_IN], F16, tag=f"ac{j % 2}", name=f"acc{j}")
                for j in range(J)
            ]
            nc.vector.tensor_scalar_mul(accs[0][:], sgn[0][:], float(deltas[0] / 2))
            for j in range(1, J):
                nc.vector.scalar_tensor_tensor(
                    accs[j][:], sgn[j][:], float(deltas[j] / 2), accs[j - 1][:],
                    op0=mybir.AluOpType.mult, op1=mybir.AluOpType.add,
                )
            qT = accs[J - 1]

            # ---- L1: preact = q0 @ W_in.T  (bias folded post-transpose) ----
            phs = [
                php.tile([128, 512], F32, tag=f"ph{fo}", name=f"ph_l1_{fo}")
                for fo in range(FO_H)
            ]
            for k in range(4):
                for fo in range(FO_H):
                    nc.tensor.matmul(
                        phs[fo][:],
                        lhsT=qT[:, 128 * k:128 * (k + 1)],
                        rhs=w1_chunk(k)[:, 512 * fo:512 * (fo + 1)],
                        start=(k == 0), stop=(k == 3),
                    )

            # ---- L2..L5 + L6 input: transpose + fused bias+relu ----
            # pre[b, f] (psum) -> raw copy to SBUF fp16 -> PE transpose ->
            # ACT relu(x + bias[f]) per [128,128] tile -> hT[f, b]
            bias_col = [b1c] + [bhc] * 4

            for layer in range(5):
                pre = act.tile([128, H], F16, tag="pre", name=f"pre{layer}")
                for fo in range(FO_H):
                    nc.vector.tensor_copy(
                        pre[:, 512 * fo:512 * (fo + 1)], phs[fo][:]
                    )
                hT = act.tile([128, H], F16, tag="hT", name=f"hT{layer}")
                for p in range(KC_H):
                    pt = ptp.tile([128, 128], F16, tag="pt", name=f"pt{layer}_{p}")
                    nc.tensor.transpose(
                        pt[:], pre[:, 128 * p:128 * (p + 1)], ident_s[:]
                    )
                    nc.scalar.activation(
                        hT[:, 128 * p:128 * (p + 1)], pt[:],
                        mybir.ActivationFunctionType.Relu,
                        bias=bias_col[layer][:, p:p + 1],
                    )
                if layer < 4:
                    phs = [
                        php.tile([128, 512], F32, tag=f"ph{fo}",
                                 name=f"ph_l{layer}_{fo}")
                        for fo in range(FO_H)
                    ]
                    for k in range(KC_H):
                        for fo in range(FO_H):
                            nc.tensor.matmul(
                                phs[fo][:],
                                lhsT=hT[:, 128 * k:128 * (k + 1)],
                                rhs=wh_chunk(k)[:, 512 * fo:512 * (fo + 1)],
                                start=(k == 0), stop=(k == KC_H - 1),
                            )

            # ---- L6: f = relu(h5 @ W_out.T + b_out) ----
            pho = php.tile([128, D_OUT], F32, tag="ph0", name="ph_l6")
            for k in range(KC_H):
                nc.tensor.matmul(
                    pho[:], lhsT=hT[:, 128 * k:128 * (k + 1)], rhs=wo_chunk(k),
                    start=(k == 0), stop=False,
                )
            nc.tensor.matmul(
                pho[:], lhsT=ones_s[:], rhs=bos[:], start=False, stop=True
            )
            fo_s = act.tile([128, D_OUT], F32, tag="fout")
            nc.scalar.activation(
                fo_s[:], pho[:], mybir.ActivationFunctionType.Relu
            )
            nc.sync.dma_start(out=out_d[:], in_=fo_s[:])

    nc.compile()
    return nc


_PROGRAM_CACHE: dict[bytes, object] = {}


def _get_program(thetas: np.ndarray, region_ids: np.ndarray):
    key = thetas.tobytes() + region_ids.tobytes()
    if key not in _PROGRAM_CACHE:
        _PROGRAM_CACHE[key] = build_program(thetas, region_ids)
    return _PROGRAM_CACHE[key]


def _host_prep(x, emb_W, W_in, b_in, W_h, b_h, W_out, b_out):
    """Shared host-side preprocessing: thresholds + weight layouts."""
    thetas, region_ids = build_vq_thresholds(emb_W)
    deltas = np.diff(region_ids).astype(np.float64)
    base_eff = float(region_ids[0]) + float(deltas.sum() / 2)

    w1t = np.ascontiguousarray(np.asarray(W_in).T).astype(np.float16)
    wht = np.ascontiguousarray(np.asarray(W_h).T).astype(np.float16)
    wot = np.ascontiguousarray(np.asarray(W_out).T).astype(np.float16)
    # layer-1 bias with the VQ base constant folded in:
    # (q0 + base) @ W_in.T + b_in = q0 @ W_in.T + (b_in + base * sum_d W_in)
    b1_eff = (
        np.asarray(b_in, dtype=np.float64)
        + base_eff * np.asarray(W_in, dtype=np.float64).sum(axis=1)
    ).astype(np.float32)
    b1c = np.ascontiguousarray(b1_eff.reshape(KC_H, 128).T)   # [128, 16]
    bhc = np.ascontiguousarray(
        np.asarray(b_h, dtype=np.float32).reshape(KC_H, 128).T
    )
    bo = np.asarray(b_out).reshape(1, D_OUT).astype(np.float16)
    return thetas, region_ids, w1t, wht, wot, b1c, bhc, bo


def make_in_maps(x, emb_W, W_in, b_in, W_h, b_h, W_out, b_out):
    x = np.asarray(x, dtype=np.float32)
    thetas, region_ids, w1t, wht, wot, b1c, bhc, bo = _host_prep(
        x, emb_W, W_in, b_in, W_h, b_h, W_out, b_out
    )
    in_maps = []
    for c in range(N_CORES):
        u, w = c // 4, c % 4
        x_core = np.ascontiguousarray(x[u::2, 128 * w:128 * (w + 1)])
        in_maps.append({
            "x": x_core, "w1t": w1t, "wht": wht, "wot": wot,
            "b1c": b1c, "bhc": bhc, "bo": bo,
        })
    return thetas, region_ids, in_maps


# --------------------------------------------------------------------------
# Entry point
# --------------------------------------------------------------------------

def kernel(x, emb_W, W_in, b_in, W_h, b_h, W_out, b_out):
    thetas, region_ids, in_maps = make_in_maps(
        x, emb_W, W_in, b_in, W_h, b_h, W_out, b_out
    )
    nc = _get_program(thetas, region_ids)
    res = run_bass_kernel_spmd(nc, in_maps, core_ids=list(range(N_CORES)))
    f = np.concatenate(
        [np.asarray(res.results[c]["out"]) for c in range(N_CORES)], axis=0
    ).astype(np.float32)
    return (f, np.float32(0.0))
